# revision 14
# baseline (speedup 1.0000x reference)
"""GraphSAGE 2-layer encoder on 8 Trainium2 NeuronCores (Bass/Tile).

v2 strategy (vs v1 baseline at ~1.18ms):
  The v1 profile showed the Vector engine as the bottleneck (952us active,
  81%): one DVE tensor_scalar per 128-edge chunk built a weighted one-hot
  "sel" matrix (1800 builds/core). v2 eliminates almost all of that:

  - Fixed-slot aggregation: every dst node gets D0=8 gather slots per
    source-half. Slot->dst mapping is then compile-time constant, so the
    selection matrices are EIGHT constant [128,128] one-hots built once.
    Unused slots gather a dedicated zero row appended to the table.
    Degree normalization (1/deg) is applied afterwards as a per-partition
    scale in the scalar-engine PSUM->SBUF copy (agg kept node-major).
  - Overflow edges (per-(dst,half) degree > 8, ~14% of edges) go through
    per-(subtile,half) chunks with DVE-built unweighted one-hots
    (~200 DVE ops/layer instead of 1800 weighted ones).
  - Layer-2 "project-first": y2 = h @ W2[256:] is computed per-core
    ([*,128] bf16), AllGathered instead of h (halves collective bytes),
    and mean-aggregated directly: agg2@W2b == (M2 h)@W2b == M2 y2.
    The output PSUM accumulates self-term + identity*agg2_scaled + bias
    with no transposes.
  - Gathers are issued per (supertile-pair, half) with ~5120 indices each
    (the SWDGE ring holds num_idxs/16+1 entries, cap 2048), cutting Q7
    prep overhead ~4x vs v1's 300 gathers.

  Nodes are range-sharded (6250/core, padded to 6400); per-edge source
  rows are fetched with gpsimd dma_gather (int16 indices -> tables split
  into two <32768-row halves, each with a trailing zero row).
"""

import dataclasses

import numpy as np
import ml_dtypes

import concourse.bass as bass
import concourse.mybir as mybir
import concourse.tile as tile
from concourse import bacc
from concourse.bass_utils import run_bass_kernel_spmd
from concourse.masks import make_identity

BF16 = ml_dtypes.bfloat16

# problem constants (hardcoded per contract)
N = 50000
E = 800000
IN_DIM = 128
HID = 256
OUT_DIM = 128

NCORES = 8
NPC = N // NCORES          # 6250 nodes per core
ST = 256                   # supertile (dst nodes per outer iteration)
NST = 25                   # supertiles per core (6400 padded rows)
NPAD = NST * ST            # 6400
P = 128

D0 = 8                     # main slots per (dst, source-half)
MAINCH = 16                # main chunks per (st, half): 2 subs x 8
HALF1 = N // 2             # 25000 rows per x-table half
TB1 = HALF1 + 1            # +1 zero row
BLK = NPC // 2             # 3125 allgather data rows per core per chunk
CBLK = BLK + 1             # +1 zero row inside each core's allgather block
HALF2 = NCORES * CBLK      # 25008 rows per y2 table half
ZROW1 = 25000              # zero-row index, layer-1 x table
ZROW2 = BLK                # zero-row index, layer-2 y2 table (core 0's)
OVPAD = 200.0              # out-of-range dst sentinel for overflow pads

# gather groups: supertile 0 alone, then pairs (1,2)...(23,24)
GROUPS = [[0]] + [[2 * i + 1, 2 * i + 2] for i in range(12)]

_PROGRAM_CACHE: dict = {}


# ----------------------------------------------------------------------------
# host-side preprocessing
# ----------------------------------------------------------------------------

def _pack_layer(src, dst, rel, half, zrow):
    """Build fixed-slot gather tables for one layer.

    rel: int16 table row (within half) per edge's src; half: 0/1 per edge.
    Returns (OVC, idx_tabs, dstov_tabs, invdeg_tabs); per core:
      idx_tab [128, NST*2*NCH*8] int16   (NCH = 16 + 2*OVC)
      dstov   [128, NST*2*2*OVC] float32 (within-sub dst of ovf slot, or 200)
      invdeg  [128, NST*2]       float32
    Instruction-order layout: blocks (group g, half h), within a block
    supertiles ascending, chunks 0..NCH-1, 128 partitions; linear slot t
    wraps to (partition t%16, col t//16), tiled x8 to 128 partitions.
    """
    deg = np.bincount(dst, minlength=N)

    # rank of each edge within its (dst, half) bucket
    key = dst * 2 + half
    order = np.argsort(key, kind="stable")
    counts = np.bincount(key, minlength=2 * N)
    starts = np.concatenate([[0], np.cumsum(counts)])
    rank = np.empty(E, dtype=np.int64)
    rank[order] = np.arange(E) - starts[key[order]]

    core = dst // NPC
    q = dst - core * NPC
    st = q >> 8
    i = q & 255
    sub = i >> 7
    ii = i & 127

    is_main = rank < D0
    # ---- main slots ----
    m_chunk = sub[is_main] * D0 + (ii[is_main] >> 4)        # 0..15
    m_part = (ii[is_main] & 15) * D0 + rank[is_main]        # 0..127
    m_core, m_st, m_half = core[is_main], st[is_main], half[is_main]
    m_rel = rel[is_main]

    # ---- overflow ----
    ov = ~is_main
    ogid = ((core[ov] * NST + st[ov]) * 2 + sub[ov]) * 2 + half[ov]
    ngroups = NCORES * NST * 4
    oorder = np.argsort(ogid, kind="stable")
    ocounts = np.bincount(ogid, minlength=ngroups)
    OVC = max(1, int(-(-ocounts.max() // P)))
    ostarts = np.concatenate([[0], np.cumsum(ocounts)])
    opos = np.empty(ov.sum(), dtype=np.int64)
    opos[oorder] = np.arange(ov.sum()) - ostarts[ogid[oorder]]
    o_chunk = MAINCH + sub[ov] * OVC + (opos >> 7)          # within (st,half)
    o_part = opos & 127
    o_core, o_st, o_half = core[ov], st[ov], half[ov]
    o_rel, o_ii = rel[ov], ii[ov]

    NCH = MAINCH + 2 * OVC

    # idx array in (core, st, half, chunk, part) space, default zero row
    idx = np.full((NCORES, NST, 2, NCH, P), zrow, dtype=np.int16)
    idx[m_core, m_st, m_half, m_chunk, m_part] = m_rel
    idx[o_core, o_st, o_half, o_chunk, o_part] = o_rel

    # [core, part, st, sub, half, ovc]; (o_chunk-MAINCH)//OVC == sub
    dstov = np.full((NCORES, P, NST, 2, 2, OVC), OVPAD, dtype=np.float32)
    dstov[o_core, o_part, o_st, (o_chunk - MAINCH) // OVC,
          o_half, (o_chunk - MAINCH) % OVC] = o_ii

    invdeg = np.zeros((NCORES, P, NST, 2), dtype=np.float32)
    nodes = np.arange(N)
    invdeg[nodes // NPC, (nodes % NPC) & 127, (nodes % NPC) >> 8,
           ((nodes % NPC) & 255) >> 7] = 1.0 / np.maximum(deg, 1.0)

    # reorder idx into instruction-order column layout with 16-partition wrap
    idx_tabs, dstov_tabs, invdeg_tabs = [], [], []
    for c in range(NCORES):
        blocks = []
        for g in GROUPS:
            for h in range(2):
                # [len(g), NCH, P] -> linear t = (st_loc*NCH + ch)*128 + p
                lin = idx[c, g, h].reshape(-1)               # len(g)*NCH*128
                w = lin.reshape(-1, 16).T                     # [16, t//16]
                blocks.append(w)
        it = np.concatenate(blocks, axis=1)                   # [16, total]
        idx_tabs.append(np.ascontiguousarray(np.tile(it, (8, 1))))
        dstov_tabs.append(np.ascontiguousarray(
            dstov[c].reshape(P, NST * 2 * 2 * OVC)))
        invdeg_tabs.append(np.ascontiguousarray(
            invdeg[c].reshape(P, NST * 2)))
    return OVC, idx_tabs, dstov_tabs, invdeg_tabs


def _preprocess(x, W1, b1, W2, b2, es0, ed0, es1, ed1):
    x = np.asarray(x, dtype=np.float32)
    es0 = np.asarray(es0, dtype=np.int64)
    ed0 = np.asarray(ed0, dtype=np.int64)
    es1 = np.asarray(es1, dtype=np.int64)
    ed1 = np.asarray(ed1, dtype=np.int64)

    # layer 1: table = x split in two halves, each with a trailing zero row
    OVC1, idx1, dv1, id1 = _pack_layer(
        es0, ed0, (es0 % HALF1).astype(np.int16), es0 // HALF1, ZROW1)

    # layer 2: y2 table rows: node n=c*NPC+j -> half j//BLK, row c*BLK+j%BLK
    c_of = es1 // NPC
    j_of = es1 % NPC
    OVC2, idx2, dv2, id2 = _pack_layer(
        es1, ed1, (c_of * CBLK + (j_of % BLK)).astype(np.int16), j_of // BLK, ZROW2)

    x_bf = x.astype(BF16)
    xtab2 = np.zeros((2 * TB1, IN_DIM), dtype=BF16)
    xtab2[0:HALF1] = x_bf[0:HALF1]
    xtab2[TB1:TB1 + HALF1] = x_bf[HALF1:N]

    xown = []
    for c in range(NCORES):
        xo = np.zeros((NPAD, IN_DIM), dtype=BF16)
        xo[:NPC] = x_bf[c * NPC:(c + 1) * NPC]
        xown.append(xo)

    W1_bf = np.asarray(W1, np.float32).astype(BF16)           # [256, 256]
    W2_bf = np.asarray(W2, np.float32).astype(BF16)           # [512, 128]
    b1_2 = np.asarray(b1, np.float32).reshape(2, P).T.copy()  # [128, 2]
    b2_r = np.asarray(b2, np.float32).reshape(1, P).astype(BF16)

    # scol[p, c] = c*16 + p//8 (selection one-hot compare values)
    scol = (np.arange(P)[:, None] // D0
            + np.arange(8)[None, :] * 16.0).astype(np.float32)

    in_maps = []
    for c in range(NCORES):
        in_maps.append({
            "xtab2": xtab2,
            "xown": xown[c],
            "w1": W1_bf,
            "w2": W2_bf,
            "b1": b1_2,
            "b2r": b2_r,
            "scol": scol,
            "idx1": idx1[c], "dv1": dv1[c], "id1": id1[c],
            "idx2": idx2[c], "dv2": dv2[c], "id2": id2[c],
        })
    return OVC1, OVC2, in_maps


# ----------------------------------------------------------------------------
# device program
# ----------------------------------------------------------------------------

def build_program(OVC1, OVC2, ablate=()):
    key = (OVC1, OVC2, tuple(sorted(ablate)))
    if key in _PROGRAM_CACHE:
        return _PROGRAM_CACHE[key]

    NCH1 = MAINCH + 2 * OVC1
    NCH2 = MAINCH + 2 * OVC2
    dt = mybir.dt
    AF = mybir.ActivationFunctionType
    nc = bacc.Bacc("TRN2", target_bir_lowering=False, debug=False,
                   num_devices=NCORES, num_swdge_queues=4,
                   dynamic_dma_scratch_size=16384)

    t_xtab2 = nc.dram_tensor("xtab2", [2 * TB1, IN_DIM], dt.bfloat16, kind="ExternalInput")
    t_xown = nc.dram_tensor("xown", [NPAD, IN_DIM], dt.bfloat16, kind="ExternalInput")
    t_w1 = nc.dram_tensor("w1", [HID, HID], dt.bfloat16, kind="ExternalInput")
    t_w2 = nc.dram_tensor("w2", [2 * HID, OUT_DIM], dt.bfloat16, kind="ExternalInput")
    t_b1 = nc.dram_tensor("b1", [P, 2], dt.float32, kind="ExternalInput")
    t_b2r = nc.dram_tensor("b2r", [1, OUT_DIM], dt.bfloat16, kind="ExternalInput")
    t_scol = nc.dram_tensor("scol", [P, 8], dt.float32, kind="ExternalInput")
    t_idx1 = nc.dram_tensor("idx1", [P, NST * 2 * NCH1 * 8], dt.int16, kind="ExternalInput")
    t_dv1 = nc.dram_tensor("dv1", [P, NST * 2 * 2 * OVC1], dt.float32, kind="ExternalInput")
    t_id1 = nc.dram_tensor("id1", [P, NST * 2], dt.float32, kind="ExternalInput")
    t_idx2 = nc.dram_tensor("idx2", [P, NST * 2 * NCH2 * 8], dt.int16, kind="ExternalInput")
    t_dv2 = nc.dram_tensor("dv2", [P, NST * 2 * 2 * OVC2], dt.float32, kind="ExternalInput")
    t_id2 = nc.dram_tensor("id2", [P, NST * 2], dt.float32, kind="ExternalInput")
    t_out = nc.dram_tensor("out", [NPAD, OUT_DIM], dt.float32, kind="ExternalOutput")

    qctr = [0]

    def gather(gat_h, in_ap, idx_sb, col0, ncols, num_idxs, ew):
        nc.gpsimd.dma_gather(
            out_ap=gat_h,
            in_ap=in_ap,
            idxs_ap=idx_sb[:, col0:col0 + ncols],
            num_idxs=num_idxs,
            num_idxs_reg=num_idxs,
            elem_size=ew,
            elem_step=P if ew != P else None,
            queue_num=qctr[0] % 4,
        )
        qctr[0] += 1

    with tile.TileContext(nc) as tc:
        with tc.tile_pool(name="const", bufs=1) as cp, \
             tc.tile_pool(name="dram", bufs=1, space="DRAM") as dp:

            # ---- constants / persistent SBUF ----
            ident_bf = cp.tile([P, P], dt.bfloat16, name="ident_bf")
            make_identity(nc, ident_bf)
            iota_i = cp.tile([P, P], dt.int32, name="iota_i")
            nc.gpsimd.iota(iota_i, pattern=[[1, P]], base=0, channel_multiplier=0)
            iota_bf = cp.tile([P, P], dt.bfloat16, name="iota_bf")
            nc.vector.tensor_copy(iota_bf[:], iota_i[:])
            ones_1 = cp.tile([1, P], dt.bfloat16, name="ones_1")
            nc.vector.memset(ones_1[:], 1.0)
            zrow_bf = cp.tile([1, P], dt.bfloat16, name="zrow_bf")
            nc.vector.memset(zrow_bf[:], 0.0)
            b2r_sb = cp.tile([1, OUT_DIM], dt.bfloat16, name="b2r_sb")

            w1_sb = cp.tile([P, 2, HID], dt.bfloat16, name="w1_sb")
            nc.sync.dma_start(w1_sb[:], t_w1.ap().rearrange("(a p) h -> p a h", p=P))
            w2_sb = cp.tile([P, 4, OUT_DIM], dt.bfloat16, name="w2_sb")
            nc.sync.dma_start(w2_sb[:], t_w2.ap().rearrange("(a p) h -> p a h", p=P))
            b1_sb = cp.tile([P, 2], dt.float32, name="b1_sb")
            nc.sync.dma_start(b1_sb[:], t_b1.ap()[:])
            nc.sync.dma_start(b2r_sb[:], t_b2r.ap()[:])
            scol_sb = cp.tile([P, 8], dt.float32, name="scol_sb")
            nc.sync.dma_start(scol_sb[:], t_scol.ap()[:])

            idx1_sb = cp.tile([P, NST * 2 * NCH1 * 8], dt.int16, name="idx1_sb")
            nc.sync.dma_start(idx1_sb[:], t_idx1.ap()[:])
            dv1_sb = cp.tile([P, NST * 2 * 2 * OVC1], dt.float32, name="dv1_sb")
            nc.sync.dma_start(dv1_sb[:], t_dv1.ap()[:])
            id1_sb = cp.tile([P, NST * 2], dt.float32, name="id1_sb")
            nc.sync.dma_start(id1_sb[:], t_id1.ap()[:])
            idx2_sb = cp.tile([P, NST * 2 * NCH2 * 8], dt.int16, name="idx2_sb")
            nc.sync.dma_start(idx2_sb[:], t_idx2.ap()[:])
            dv2_sb = cp.tile([P, NST * 2 * 2 * OVC2], dt.float32, name="dv2_sb")
            nc.sync.dma_start(dv2_sb[:], t_dv2.ap()[:])
            id2_sb = cp.tile([P, NST * 2], dt.float32, name="id2_sb")
            nc.sync.dma_start(id2_sb[:], t_id2.ap()[:])

            # 8 constant selection one-hots: sel8[c][s, d] = (d == c*16 + s//8)
            sel8 = cp.tile([P, 8, P], dt.bfloat16, name="sel8")
            for c in range(8):
                nc.vector.tensor_scalar(
                    out=sel8[:, c, :], in0=iota_bf[:],
                    scalar1=scol_sb[:, c:c + 1], scalar2=None,
                    op0=mybir.AluOpType.is_equal,
                )

            # persistent transposed h (self-features for layer 2)
            hta = cp.tile([P, NPAD], dt.bfloat16, name="hta")
            htb = cp.tile([P, NPAD], dt.bfloat16, name="htb")

            # allgather buffers (y2 = h @ W2b, project-first); each core's
            # send block carries a trailing zero row so the gathered table
            # gets its pad target without a second writer on the Shared side
            y2own = dp.tile([2 * CBLK, OUT_DIM], dt.bfloat16, name="y2own")
            y2full = [
                dp.tile([HALF2, OUT_DIM], dt.bfloat16, name=f"y2full{k}",
                        addr_space="Shared")
                for k in range(2)
            ]
            for k in range(2):
                nc.sync.dma_start(y2own[(k + 1) * CBLK - 1:(k + 1) * CBLK, :],
                                  zrow_bf[:])

            # ---- layer 1 ----
            idx_off = [0]

            def layer(lnum, NCH, OVC, idx_sb, dv_sb, id_sb, in_aps, ew):
                """Emit one layer's supertile loop. Returns nothing."""
                with tc.tile_pool(name=f"l{lnum}sb", bufs=2) as sp, \
                     tc.tile_pool(name=f"l{lnum}ps", bufs=2, space="PSUM") as pp:
                    off = 0
                    for g in GROUPS:
                        nst_g = len(g)
                        nidx = nst_g * NCH * P
                        ncols = nidx // 16
                        tag = f"gat{lnum}_{nst_g}"
                        gat = sp.tile([P, 2, nst_g * NCH, ew], dt.bfloat16,
                                      name=tag, tag=tag,
                                      bufs=3 if ew == P else (2 if nst_g > 1 else 1))
                        for h in range(2):
                            if "gather" in ablate:
                                nc.vector.memset(gat[:, h], 0.0)
                                continue
                            # HW caps one dma_gather at 1024 indices
                            # (16 engines x 64-descriptor rings); 768-idx
                            # instructions leave ring slack so consecutive
                            # gathers on a queue pipeline
                            nch_all = nst_g * NCH
                            for w0 in range(0, nch_all, 6):
                                wn = min(6, nch_all - w0)
                                gather(gat[:, h, w0:w0 + wn, :], in_aps[h],
                                       idx_sb, off + h * ncols + w0 * 8,
                                       wn * 8, wn * P, ew)
                        for sti, st in enumerate(g):
                            co = sti * NCH
                            if lnum == 1:
                                _l1_supertile(st, gat, co, OVC, dv_sb, id_sb, sp, pp)
                            else:
                                _l2_supertile(st, gat, co, OVC, dv_sb, id_sb, sp, pp)
                        off += 2 * ncols
                        if lnum == 1 and g[-1] == 12:
                            _allgather(0)
                    if lnum == 1:
                        _allgather(1)

            def _allgather(k):
                nc.gpsimd.collective_compute(
                    "AllGather",
                    mybir.AluOpType.bypass,
                    replica_groups=[list(range(NCORES))],
                    ins=[y2own[k * CBLK:(k + 1) * CBLK, :].opt()],
                    outs=[y2full[k][:].opt()],
                )

            def _agg_psum(gat, co, sub, OVC, dv_sb, st, pp, sp):
                """Accumulate main+overflow into a [128dst, 128f] PSUM tile."""
                agg_ps = pp.tile([P, P], dt.float32, name="agg_ps",
                                 tag="agg_ps", bufs=2)
                for c in range(8):
                    for h in range(2):
                        nc.tensor.matmul(
                            agg_ps[:],
                            lhsT=sel8[:, c, :],
                            rhs=gat[:, h, co + sub * 8 + c, 0:P],
                            start=(c == 0 and h == 0), stop=False)
                n_ov = 2 * OVC
                kk = 0
                for h in range(2):
                    for o in range(OVC):
                        col = ((st * 2 + sub) * 2 + h) * OVC + o
                        if "sel" in ablate:
                            sel = sel8[:, 0, :]
                        else:
                            sel = sp.tile([P, P], dt.bfloat16, name="sel_ov",
                                          tag="sel_ov", bufs=8)
                            nc.vector.tensor_scalar(
                                out=sel[:], in0=iota_bf[:],
                                scalar1=dv_sb[:, col:col + 1], scalar2=None,
                                op0=mybir.AluOpType.is_equal,
                            )
                        nc.tensor.matmul(
                            agg_ps[:],
                            lhsT=sel if "sel" in ablate else sel[:],
                            rhs=gat[:, h, co + MAINCH + sub * OVC + o, 0:P],
                            start=False, stop=(kk == n_ov - 1))
                        kk += 1
                return agg_ps

            def _l1_supertile(st, gat, co, OVC, dv_sb, id_sb, sp, pp):
                r0 = st * ST
                # own features, transposed (self term)
                xo = sp.tile([P, 2, IN_DIM], dt.bfloat16, name="xo", tag="xo", bufs=3)
                nc.sync.dma_start(
                    xo[:], t_xown.ap()[r0:r0 + ST, :].rearrange("(a p) f -> p a f", p=P))
                xT_ps = pp.tile([P, ST], dt.bfloat16, name="xT_ps", tag="xT_ps", bufs=1)
                nc.tensor.transpose(xT_ps[:, 0:P], xo[:, 0, :], ident_bf[:])
                nc.tensor.transpose(xT_ps[:, P:ST], xo[:, 1, :], ident_bf[:])
                xT = sp.tile([P, ST], dt.bfloat16, name="xT", tag="xT")
                nc.scalar.activation(xT[:], xT_ps[:], AF.Copy)

                # aggregation, node-major, then scale by 1/deg and transpose
                aggT_ps = pp.tile([P, ST], dt.bfloat16, name="aggT_ps", tag="aggT_ps", bufs=1)
                for sub in range(2):
                    agg_ps = _agg_psum(gat, co, sub, OVC, dv_sb, st, pp, sp)
                    agg_sb = sp.tile([P, P], dt.bfloat16, name="agg_sb",
                                     tag="agg_sb", bufs=3)
                    nc.scalar.activation(
                        agg_sb[:], agg_ps[:], AF.Copy,
                        scale=id_sb[:, st * 2 + sub:st * 2 + sub + 1])
                    nc.tensor.transpose(aggT_ps[:, sub * P:(sub + 1) * P],
                                        agg_sb[:], ident_bf[:])
                aggT = sp.tile([P, ST], dt.bfloat16, name="aggT", tag="aggT")
                nc.scalar.activation(aggT[:], aggT_ps[:], AF.Copy)

                # hT = relu(W1^T @ [x; agg] + b1), two hid halves
                for hh, hstore in ((0, hta), (1, htb)):
                    hT_ps = pp.tile([P, ST], dt.float32, name="hT_ps", tag="hT_ps")
                    nc.tensor.matmul(hT_ps[:], lhsT=w1_sb[:, 0, hh * P:(hh + 1) * P],
                                     rhs=xT[:], start=True, stop=False)
                    nc.tensor.matmul(hT_ps[:], lhsT=w1_sb[:, 1, hh * P:(hh + 1) * P],
                                     rhs=aggT[:], start=False, stop=True)
                    nc.scalar.activation(hstore[:, r0:r0 + ST], hT_ps[:],
                                         AF.Relu, bias=b1_sb[:, hh:hh + 1])

                # y2 = h @ W2b rows -> y2own (project-first for layer 2)
                y2T_ps = pp.tile([P, ST], dt.float32, name="y2T_ps", tag="y2T_ps", bufs=1)
                nc.tensor.matmul(y2T_ps[:], lhsT=w2_sb[:, 2, :],
                                 rhs=hta[:, r0:r0 + ST], start=True, stop=False)
                nc.tensor.matmul(y2T_ps[:], lhsT=w2_sb[:, 3, :],
                                 rhs=htb[:, r0:r0 + ST], start=False, stop=True)
                y2T = sp.tile([P, ST], dt.bfloat16, name="y2T", tag="y2T")
                nc.scalar.activation(y2T[:], y2T_ps[:], AF.Copy)
                for nh in range(2):
                    rr = r0 + nh * P
                    nrows = min(P, NPC - rr)
                    if nrows <= 0:
                        continue
                    y2r_ps = pp.tile([P, P], dt.bfloat16, name="y2r_ps",
                                     tag="y2r_ps", bufs=1)
                    nc.tensor.transpose(y2r_ps[:], y2T[:, nh * P:(nh + 1) * P],
                                        ident_bf[:])
                    y2r = sp.tile([P, P], dt.bfloat16, name="y2r", tag="y2r", bufs=3)
                    nc.scalar.activation(y2r[:], y2r_ps[:], AF.Copy)
                    # dest row for node j: j + (1 if j >= BLK else 0)
                    if rr + nrows <= BLK:
                        nc.sync.dma_start(y2own[rr:rr + nrows, :], y2r[0:nrows, :])
                    elif rr >= BLK:
                        nc.sync.dma_start(y2own[rr + 1:rr + 1 + nrows, :],
                                          y2r[0:nrows, :])
                    else:
                        n1 = BLK - rr
                        nc.sync.dma_start(y2own[rr:BLK, :], y2r[0:n1, :])
                        nc.sync.dma_start(y2own[BLK + 1:rr + 1 + nrows, :],
                                          y2r[n1:nrows, :])

            def _l2_supertile(st, gat, co, OVC, dv_sb, id_sb, sp, pp):
                r0 = st * ST
                for sub in range(2):
                    rr = r0 + sub * P
                    agg_ps = _agg_psum(gat, co, sub, OVC, dv_sb, st, pp, sp)
                    agg_sb = sp.tile([P, P], dt.bfloat16, name="agg2_sb",
                                     tag="agg2_sb", bufs=3)
                    nc.scalar.activation(
                        agg_sb[:], agg_ps[:], AF.Copy,
                        scale=id_sb[:, st * 2 + sub:st * 2 + sub + 1])
                    out_ps = pp.tile([P, OUT_DIM], dt.float32, name="out_ps",
                                     tag="out_ps")
                    nc.tensor.matmul(out_ps[:], lhsT=hta[:, rr:rr + P],
                                     rhs=w2_sb[:, 0, :], start=True, stop=False)
                    nc.tensor.matmul(out_ps[:], lhsT=htb[:, rr:rr + P],
                                     rhs=w2_sb[:, 1, :], start=False, stop=False)
                    nc.tensor.matmul(out_ps[:], lhsT=ident_bf[:],
                                     rhs=agg_sb[:], start=False, stop=False)
                    nc.tensor.matmul(out_ps[:], lhsT=ones_1[:],
                                     rhs=b2r_sb[:], start=False, stop=True)
                    o_sb = sp.tile([P, OUT_DIM], dt.float32, name="o_sb",
                                   tag="o_sb", bufs=3)
                    nc.scalar.activation(o_sb[:], out_ps[:], AF.Relu)
                    nc.sync.dma_start(t_out.ap()[rr:rr + P, :], o_sb[:])

            def wide_view(ap):
                # overlapping 512B-per-descriptor window: each idx fetches
                # its 256B row plus the next row (junk, sliced off later) --
                # sub-512B descriptors cost ~2x per descriptor on the SDMA
                return dataclasses.replace(
                    ap, ap=[[P, ap.ap[0][1] - 1], [1, 2 * P]])

            layer(1, NCH1, OVC1, idx1_sb, dv1_sb, id1_sb,
                  [t_xtab2.ap()[h * TB1:h * TB1 + TB1, :] for h in range(2)],
                  P)
            layer(2, NCH2, OVC2, idx2_sb, dv2_sb, id2_sb,
                  [wide_view(y2full[h][:]) for h in range(2)], 2 * P)

    nc.compile()
    _PROGRAM_CACHE[key] = nc
    return nc


# ----------------------------------------------------------------------------
# entry point
# ----------------------------------------------------------------------------

def kernel(x, W1, b1, W2, b2, edge_src0, edge_dst0, edge_src1, edge_dst1,
           _want_results=False, **_ignored):
    OVC1, OVC2, in_maps = _preprocess(x, W1, b1, W2, b2,
                                      edge_src0, edge_dst0, edge_src1, edge_dst1)
    nc = build_program(OVC1, OVC2)
    res = run_bass_kernel_spmd(nc, in_maps, core_ids=list(range(NCORES)))
    out = np.concatenate([res.results[c]["out"][:NPC] for c in range(NCORES)], axis=0)
    out = np.ascontiguousarray(out, dtype=np.float32)
    if _want_results:
        return out, res
    return out


# revision 16
# speedup vs baseline: 1.9697x; 1.9697x over previous
"""GraphSAGE 2-layer encoder on 8 Trainium2 NeuronCores (Bass/Tile).

v2 strategy (vs v1 baseline at ~1.18ms):
  The v1 profile showed the Vector engine as the bottleneck (952us active,
  81%): one DVE tensor_scalar per 128-edge chunk built a weighted one-hot
  "sel" matrix (1800 builds/core). v2 eliminates almost all of that:

  - Fixed-slot aggregation: every dst node gets D0=8 gather slots per
    source-half. Slot->dst mapping is then compile-time constant, so the
    selection matrices are EIGHT constant [128,128] one-hots built once.
    Unused slots gather a dedicated zero row appended to the table.
    Degree normalization (1/deg) is applied afterwards as a per-partition
    scale in the scalar-engine PSUM->SBUF copy (agg kept node-major).
  - Overflow edges (per-(dst,half) degree > 8, ~14% of edges) go through
    per-(subtile,half) chunks with DVE-built unweighted one-hots
    (~200 DVE ops/layer instead of 1800 weighted ones).
  - Layer-2 "project-first": y2 = h @ W2[256:] is computed per-core
    ([*,128] bf16), AllGathered instead of h (halves collective bytes),
    and mean-aggregated directly: agg2@W2b == (M2 h)@W2b == M2 y2.
    The output PSUM accumulates self-term + identity*agg2_scaled + bias
    with no transposes.
  - Gathers are issued per (supertile-pair, half) with ~5120 indices each
    (the SWDGE ring holds num_idxs/16+1 entries, cap 2048), cutting Q7
    prep overhead ~4x vs v1's 300 gathers.

  Nodes are range-sharded (6250/core, padded to 6400); per-edge source
  rows are fetched with gpsimd dma_gather (int16 indices -> tables split
  into two <32768-row halves, each with a trailing zero row).
"""

import dataclasses

import numpy as np
import ml_dtypes

import concourse.bass as bass
import concourse.mybir as mybir
import concourse.tile as tile
from concourse import bacc
from concourse.bass_utils import run_bass_kernel_spmd
from concourse.masks import make_identity

BF16 = ml_dtypes.bfloat16

# problem constants (hardcoded per contract)
N = 50000
E = 800000
IN_DIM = 128
HID = 256
OUT_DIM = 128

NCORES = 8
NPC = N // NCORES          # 6250 nodes per core
ST = 256                   # supertile (dst nodes per outer iteration)
NST = 25                   # supertiles per core (6400 padded rows)
NPAD = NST * ST            # 6400
P = 128

D0 = 8                     # main slots per (dst, source-half)
MAINCH = 16                # main chunks per (st, half): 2 subs x 8
HALF1 = N // 2             # 25000 rows per x-table half
BLK = NPC // 2             # 3125 allgather data rows per core per chunk
NZ = 16                    # zero rows appended per core allgather block
CBLK = BLK + NZ            # 3141
HALF2 = NCORES * CBLK      # 25128 rows per y2 table half
NZ1 = 128                  # zero rows appended per layer-1 x-table half
TB1 = HALF1 + NZ1          # 25128
OVPAD = 200.0              # out-of-range dst sentinel for overflow pads

# gather groups: supertile 0 alone, then pairs (1,2)...(23,24)
GROUPS = [[0]] + [[2 * i + 1, 2 * i + 2] for i in range(12)]

_PROGRAM_CACHE: dict = {}


# ----------------------------------------------------------------------------
# host-side preprocessing
# ----------------------------------------------------------------------------

def _pack_layer(src, dst, rel, half, zrows):
    """Build fixed-slot gather tables for one layer.

    rel: int16 table row (within half) per edge's src; half: 0/1 per edge.
    Returns (OVC, idx_tabs, dstov_tabs, invdeg_tabs); per core:
      idx_tab [128, NST*2*NCH*8] int16   (NCH = 16 + 2*OVC)
      dstov   [128, NST*2*2*OVC] float32 (within-sub dst of ovf slot, or 200)
      invdeg  [128, NST*2]       float32
    Instruction-order layout: blocks (group g, half h), within a block
    supertiles ascending, chunks 0..NCH-1, 128 partitions; linear slot t
    wraps to (partition t%16, col t//16), tiled x8 to 128 partitions.
    """
    deg = np.bincount(dst, minlength=N)

    # rank of each edge within its (dst, half) bucket
    key = dst * 2 + half
    order = np.argsort(key, kind="stable")
    counts = np.bincount(key, minlength=2 * N)
    starts = np.concatenate([[0], np.cumsum(counts)])
    rank = np.empty(E, dtype=np.int64)
    rank[order] = np.arange(E) - starts[key[order]]

    core = dst // NPC
    q = dst - core * NPC
    st = q >> 8
    i = q & 255
    sub = i >> 7
    ii = i & 127

    is_main = rank < D0
    # ---- main slots ----
    m_chunk = sub[is_main] * D0 + (ii[is_main] >> 4)        # 0..15
    m_part = (ii[is_main] & 15) * D0 + rank[is_main]        # 0..127
    m_core, m_st, m_half = core[is_main], st[is_main], half[is_main]
    m_rel = rel[is_main]

    # ---- overflow ----
    ov = ~is_main
    ogid = ((core[ov] * NST + st[ov]) * 2 + sub[ov]) * 2 + half[ov]
    ngroups = NCORES * NST * 4
    oorder = np.argsort(ogid, kind="stable")
    ocounts = np.bincount(ogid, minlength=ngroups)
    OVC = max(1, int(-(-ocounts.max() // P)))
    ostarts = np.concatenate([[0], np.cumsum(ocounts)])
    opos = np.empty(ov.sum(), dtype=np.int64)
    opos[oorder] = np.arange(ov.sum()) - ostarts[ogid[oorder]]
    o_chunk = MAINCH + sub[ov] * OVC + (opos >> 7)          # within (st,half)
    o_part = opos & 127
    o_core, o_st, o_half = core[ov], st[ov], half[ov]
    o_rel, o_ii = rel[ov], ii[ov]

    NCH = MAINCH + 2 * OVC

    # idx array in (core, st, half, chunk, part) space; pads spread over
    # many distinct zero rows so they don't hotspot one HBM line
    idx = np.ascontiguousarray(np.broadcast_to(
        zrows.astype(np.int16), (NCORES, NST, 2, NCH, P)))
    idx[m_core, m_st, m_half, m_chunk, m_part] = m_rel
    idx[o_core, o_st, o_half, o_chunk, o_part] = o_rel

    # [core, part, st, sub, half, ovc]; (o_chunk-MAINCH)//OVC == sub
    dstov = np.full((NCORES, P, NST, 2, 2, OVC), OVPAD, dtype=np.float32)
    dstov[o_core, o_part, o_st, (o_chunk - MAINCH) // OVC,
          o_half, (o_chunk - MAINCH) % OVC] = o_ii

    invdeg = np.zeros((NCORES, P, NST, 2), dtype=np.float32)
    nodes = np.arange(N)
    invdeg[nodes // NPC, (nodes % NPC) & 127, (nodes % NPC) >> 8,
           ((nodes % NPC) & 255) >> 7] = 1.0 / np.maximum(deg, 1.0)

    # reorder idx into instruction-order column layout with 16-partition wrap
    idx_tabs, dstov_tabs, invdeg_tabs = [], [], []
    for c in range(NCORES):
        blocks = []
        for g in GROUPS:
            for h in range(2):
                # [len(g), NCH, P] -> linear t = (st_loc*NCH + ch)*128 + p
                lin = idx[c, g, h].reshape(-1)               # len(g)*NCH*128
                w = lin.reshape(-1, 16).T                     # [16, t//16]
                blocks.append(w)
        it = np.concatenate(blocks, axis=1)                   # [16, total]
        idx_tabs.append(np.ascontiguousarray(np.tile(it, (8, 1))))
        dstov_tabs.append(np.ascontiguousarray(
            dstov[c].reshape(P, NST * 2 * 2 * OVC)))
        invdeg_tabs.append(np.ascontiguousarray(
            invdeg[c].reshape(P, NST * 2)))
    return OVC, idx_tabs, dstov_tabs, invdeg_tabs


def _preprocess(x, W1, b1, W2, b2, es0, ed0, es1, ed1):
    x = np.asarray(x, dtype=np.float32)
    es0 = np.asarray(es0, dtype=np.int64)
    ed0 = np.asarray(ed0, dtype=np.int64)
    es1 = np.asarray(es1, dtype=np.int64)
    ed1 = np.asarray(ed1, dtype=np.int64)

    # layer 1: table = x split in two halves, each with trailing zero rows
    p_ar = np.arange(P)
    zr1 = HALF1 + (p_ar % NZ1)
    OVC1, idx1, dv1, id1 = _pack_layer(
        es0, ed0, (es0 % HALF1).astype(np.int16), es0 // HALF1, zr1)

    # layer 2: y2 table rows: node n=c*NPC+j -> half j//BLK, row c*CBLK+j%BLK
    c_of = es1 // NPC
    j_of = es1 % NPC
    zr2 = (p_ar % NCORES) * CBLK + BLK + ((p_ar // NCORES) % NZ)
    OVC2, idx2, dv2, id2 = _pack_layer(
        es1, ed1, (c_of * CBLK + (j_of % BLK)).astype(np.int16), j_of // BLK, zr2)

    x_bf = x.astype(BF16)
    xtab2 = np.zeros((2 * TB1, IN_DIM), dtype=BF16)
    xtab2[0:HALF1] = x_bf[0:HALF1]
    xtab2[TB1:TB1 + HALF1] = x_bf[HALF1:N]

    xown = []
    for c in range(NCORES):
        xo = np.zeros((NPAD, IN_DIM), dtype=BF16)
        xo[:NPC] = x_bf[c * NPC:(c + 1) * NPC]
        xown.append(xo)

    W1_bf = np.asarray(W1, np.float32).astype(BF16)           # [256, 256]
    W2_bf = np.asarray(W2, np.float32).astype(BF16)           # [512, 128]
    b1_2 = np.asarray(b1, np.float32).reshape(2, P).T.copy()  # [128, 2]
    b2_r = np.asarray(b2, np.float32).reshape(1, P).astype(BF16)

    # scol[p, c] = c*16 + p//8 (selection one-hot compare values)
    scol = (np.arange(P)[:, None] // D0
            + np.arange(8)[None, :] * 16.0).astype(np.float32)

    in_maps = []
    for c in range(NCORES):
        in_maps.append({
            "xtab2": xtab2,
            "xown": xown[c],
            "w1": W1_bf,
            "w2": W2_bf,
            "b1": b1_2,
            "b2r": b2_r,
            "scol": scol,
            "idx1": idx1[c], "dv1": dv1[c], "id1": id1[c],
            "idx2": idx2[c], "dv2": dv2[c], "id2": id2[c],
        })
    return OVC1, OVC2, in_maps


# ----------------------------------------------------------------------------
# device program
# ----------------------------------------------------------------------------

def build_program(OVC1, OVC2, ablate=()):
    key = (OVC1, OVC2, tuple(sorted(ablate)))
    if key in _PROGRAM_CACHE:
        return _PROGRAM_CACHE[key]

    NCH1 = MAINCH + 2 * OVC1
    NCH2 = MAINCH + 2 * OVC2
    dt = mybir.dt
    AF = mybir.ActivationFunctionType
    nc = bacc.Bacc("TRN2", target_bir_lowering=False, debug=False,
                   num_devices=NCORES, num_swdge_queues=4,
                   dynamic_dma_scratch_size=32768)

    t_xtab2 = nc.dram_tensor("xtab2", [2 * TB1, IN_DIM], dt.bfloat16, kind="ExternalInput")
    t_xown = nc.dram_tensor("xown", [NPAD, IN_DIM], dt.bfloat16, kind="ExternalInput")
    t_w1 = nc.dram_tensor("w1", [HID, HID], dt.bfloat16, kind="ExternalInput")
    t_w2 = nc.dram_tensor("w2", [2 * HID, OUT_DIM], dt.bfloat16, kind="ExternalInput")
    t_b1 = nc.dram_tensor("b1", [P, 2], dt.float32, kind="ExternalInput")
    t_b2r = nc.dram_tensor("b2r", [1, OUT_DIM], dt.bfloat16, kind="ExternalInput")
    t_scol = nc.dram_tensor("scol", [P, 8], dt.float32, kind="ExternalInput")
    t_idx1 = nc.dram_tensor("idx1", [P, NST * 2 * NCH1 * 8], dt.int16, kind="ExternalInput")
    t_dv1 = nc.dram_tensor("dv1", [P, NST * 2 * 2 * OVC1], dt.float32, kind="ExternalInput")
    t_id1 = nc.dram_tensor("id1", [P, NST * 2], dt.float32, kind="ExternalInput")
    t_idx2 = nc.dram_tensor("idx2", [P, NST * 2 * NCH2 * 8], dt.int16, kind="ExternalInput")
    t_dv2 = nc.dram_tensor("dv2", [P, NST * 2 * 2 * OVC2], dt.float32, kind="ExternalInput")
    t_id2 = nc.dram_tensor("id2", [P, NST * 2], dt.float32, kind="ExternalInput")
    t_out = nc.dram_tensor("out", [NPAD, OUT_DIM], dt.float32, kind="ExternalOutput")

    qctr = [0]

    def gather(gat_h, in_ap, idx_sb, col0, ncols, num_idxs, ew):
        nc.gpsimd.dma_gather(
            out_ap=gat_h,
            in_ap=in_ap,
            idxs_ap=idx_sb[:, col0:col0 + ncols],
            num_idxs=num_idxs,
            num_idxs_reg=num_idxs,
            elem_size=ew,
            elem_step=P if ew != P else None,
            queue_num=qctr[0] % 4,
        )
        qctr[0] += 1

    with tile.TileContext(nc) as tc:
        with tc.tile_pool(name="const", bufs=1) as cp, \
             tc.tile_pool(name="dram", bufs=1, space="DRAM") as dp:

            # ---- constants / persistent SBUF ----
            ident_bf = cp.tile([P, P], dt.bfloat16, name="ident_bf")
            make_identity(nc, ident_bf)
            iota_i = cp.tile([P, P], dt.int32, name="iota_i")
            nc.gpsimd.iota(iota_i, pattern=[[1, P]], base=0, channel_multiplier=0)
            iota_bf = cp.tile([P, P], dt.bfloat16, name="iota_bf")
            nc.vector.tensor_copy(iota_bf[:], iota_i[:])
            ones_1 = cp.tile([1, P], dt.bfloat16, name="ones_1")
            nc.vector.memset(ones_1[:], 1.0)
            zrow_bf = cp.tile([NZ, P], dt.bfloat16, name="zrow_bf")
            nc.vector.memset(zrow_bf[:], 0.0)
            b2r_sb = cp.tile([1, OUT_DIM], dt.bfloat16, name="b2r_sb")

            w1_sb = cp.tile([P, 2, HID], dt.bfloat16, name="w1_sb")
            nc.sync.dma_start(w1_sb[:], t_w1.ap().rearrange("(a p) h -> p a h", p=P))
            w2_sb = cp.tile([P, 4, OUT_DIM], dt.bfloat16, name="w2_sb")
            nc.sync.dma_start(w2_sb[:], t_w2.ap().rearrange("(a p) h -> p a h", p=P))
            b1_sb = cp.tile([P, 2], dt.float32, name="b1_sb")
            nc.sync.dma_start(b1_sb[:], t_b1.ap()[:])
            nc.sync.dma_start(b2r_sb[:], t_b2r.ap()[:])
            scol_sb = cp.tile([P, 8], dt.float32, name="scol_sb")
            nc.sync.dma_start(scol_sb[:], t_scol.ap()[:])

            idx1_sb = cp.tile([P, NST * 2 * NCH1 * 8], dt.int16, name="idx1_sb")
            nc.sync.dma_start(idx1_sb[:], t_idx1.ap()[:])
            dv1_sb = cp.tile([P, NST * 2 * 2 * OVC1], dt.float32, name="dv1_sb")
            nc.sync.dma_start(dv1_sb[:], t_dv1.ap()[:])
            id1_sb = cp.tile([P, NST * 2], dt.float32, name="id1_sb")
            nc.sync.dma_start(id1_sb[:], t_id1.ap()[:])
            idx2_sb = cp.tile([P, NST * 2 * NCH2 * 8], dt.int16, name="idx2_sb")
            nc.sync.dma_start(idx2_sb[:], t_idx2.ap()[:])
            dv2_sb = cp.tile([P, NST * 2 * 2 * OVC2], dt.float32, name="dv2_sb")
            nc.sync.dma_start(dv2_sb[:], t_dv2.ap()[:])
            id2_sb = cp.tile([P, NST * 2], dt.float32, name="id2_sb")
            nc.sync.dma_start(id2_sb[:], t_id2.ap()[:])

            # 8 constant selection one-hots: sel8[c][s, d] = (d == c*16 + s//8)
            sel8 = cp.tile([P, 8, P], dt.bfloat16, name="sel8")
            for c in range(8):
                nc.vector.tensor_scalar(
                    out=sel8[:, c, :], in0=iota_bf[:],
                    scalar1=scol_sb[:, c:c + 1], scalar2=None,
                    op0=mybir.AluOpType.is_equal,
                )

            # persistent transposed h (self-features for layer 2)
            hta = cp.tile([P, NPAD], dt.bfloat16, name="hta")
            htb = cp.tile([P, NPAD], dt.bfloat16, name="htb")

            # allgather buffers (y2 = h @ W2b, project-first); each core's
            # send block carries a trailing zero row so the gathered table
            # gets its pad target without a second writer on the Shared side
            y2own = dp.tile([2 * CBLK, OUT_DIM], dt.bfloat16, name="y2own")
            y2full = [
                dp.tile([HALF2, OUT_DIM], dt.bfloat16, name=f"y2full{k}",
                        addr_space="Shared")
                for k in range(2)
            ]
            for k in range(2):
                nc.sync.dma_start(y2own[k * CBLK + BLK:(k + 1) * CBLK, :],
                                  zrow_bf[:])

            # ---- layer 1 ----
            idx_off = [0]

            def layer(lnum, NCH, OVC, idx_sb, dv_sb, id_sb, in_aps, ew):
                """Emit one layer's supertile loop. Returns nothing."""
                with tc.tile_pool(name=f"l{lnum}sb", bufs=2) as sp, \
                     tc.tile_pool(name=f"l{lnum}ps", bufs=2, space="PSUM") as pp:
                    off = 0
                    for g in GROUPS:
                        nst_g = len(g)
                        nidx = nst_g * NCH * P
                        ncols = nidx // 16
                        tag = f"gat{lnum}_{nst_g}"
                        gat = sp.tile([P, 2, nst_g * NCH, ew], dt.bfloat16,
                                      name=tag, tag=tag, bufs=3)
                        for h in range(2):
                            if "gather" in ablate:
                                nc.vector.memset(gat[:, h], 0.0)
                                continue
                            # HW caps one dma_gather at 1024 indices
                            # (16 engines x 64-descriptor rings); 768-idx
                            # instructions leave ring slack so consecutive
                            # gathers on a queue pipeline
                            nch_all = nst_g * NCH
                            for w0 in range(0, nch_all, 6):
                                wn = min(6, nch_all - w0)
                                gather(gat[:, h, w0:w0 + wn, :], in_aps[h],
                                       idx_sb, off + h * ncols + w0 * 8,
                                       wn * 8, wn * P, ew)
                        for sti, st in enumerate(g):
                            co = sti * NCH
                            if lnum == 1:
                                _l1_supertile(st, gat, co, OVC, dv_sb, id_sb, sp, pp)
                            else:
                                _l2_supertile(st, gat, co, OVC, dv_sb, id_sb, sp, pp)
                        off += 2 * ncols
                        if lnum == 1 and g[-1] == 12:
                            _allgather(0)
                    if lnum == 1:
                        _allgather(1)

            def _allgather(k):
                nc.gpsimd.collective_compute(
                    "AllGather",
                    mybir.AluOpType.bypass,
                    replica_groups=[list(range(NCORES))],
                    ins=[y2own[k * CBLK:(k + 1) * CBLK, :].opt()],
                    outs=[y2full[k][:].opt()],
                )

            def _agg_psum(gat, co, sub, OVC, dv_sb, st, pp, sp):
                """Accumulate main+overflow into a [128dst, 128f] PSUM tile."""
                agg_ps = pp.tile([P, P], dt.float32, name="agg_ps",
                                 tag="agg_ps", bufs=2)
                for c in range(8):
                    for h in range(2):
                        nc.tensor.matmul(
                            agg_ps[:],
                            lhsT=sel8[:, c, :],
                            rhs=gat[:, h, co + sub * 8 + c, 0:P],
                            start=(c == 0 and h == 0), stop=False)
                n_ov = 2 * OVC
                kk = 0
                for h in range(2):
                    for o in range(OVC):
                        col = ((st * 2 + sub) * 2 + h) * OVC + o
                        if "sel" in ablate:
                            sel = sel8[:, 0, :]
                        else:
                            sel = sp.tile([P, P], dt.bfloat16, name="sel_ov",
                                          tag="sel_ov", bufs=8)
                            nc.vector.tensor_scalar(
                                out=sel[:], in0=iota_bf[:],
                                scalar1=dv_sb[:, col:col + 1], scalar2=None,
                                op0=mybir.AluOpType.is_equal,
                            )
                        nc.tensor.matmul(
                            agg_ps[:],
                            lhsT=sel if "sel" in ablate else sel[:],
                            rhs=gat[:, h, co + MAINCH + sub * OVC + o, 0:P],
                            start=False, stop=(kk == n_ov - 1))
                        kk += 1
                return agg_ps

            def _l1_supertile(st, gat, co, OVC, dv_sb, id_sb, sp, pp):
                r0 = st * ST
                # own features, transposed (self term)
                xo = sp.tile([P, 2, IN_DIM], dt.bfloat16, name="xo", tag="xo", bufs=3)
                nc.sync.dma_start(
                    xo[:], t_xown.ap()[r0:r0 + ST, :].rearrange("(a p) f -> p a f", p=P))
                xT_ps = pp.tile([P, ST], dt.bfloat16, name="xT_ps", tag="xT_ps", bufs=1)
                nc.tensor.transpose(xT_ps[:, 0:P], xo[:, 0, :], ident_bf[:])
                nc.tensor.transpose(xT_ps[:, P:ST], xo[:, 1, :], ident_bf[:])
                xT = sp.tile([P, ST], dt.bfloat16, name="xT", tag="xT")
                nc.scalar.activation(xT[:], xT_ps[:], AF.Copy)

                # aggregation, node-major, then scale by 1/deg and transpose
                aggT_ps = pp.tile([P, ST], dt.bfloat16, name="aggT_ps", tag="aggT_ps", bufs=1)
                for sub in range(2):
                    agg_ps = _agg_psum(gat, co, sub, OVC, dv_sb, st, pp, sp)
                    agg_sb = sp.tile([P, P], dt.bfloat16, name="agg_sb",
                                     tag="agg_sb", bufs=3)
                    nc.scalar.activation(
                        agg_sb[:], agg_ps[:], AF.Copy,
                        scale=id_sb[:, st * 2 + sub:st * 2 + sub + 1])
                    nc.tensor.transpose(aggT_ps[:, sub * P:(sub + 1) * P],
                                        agg_sb[:], ident_bf[:])
                aggT = sp.tile([P, ST], dt.bfloat16, name="aggT", tag="aggT")
                nc.scalar.activation(aggT[:], aggT_ps[:], AF.Copy)

                # hT = relu(W1^T @ [x; agg] + b1), two hid halves
                for hh, hstore in ((0, hta), (1, htb)):
                    hT_ps = pp.tile([P, ST], dt.float32, name="hT_ps", tag="hT_ps")
                    nc.tensor.matmul(hT_ps[:], lhsT=w1_sb[:, 0, hh * P:(hh + 1) * P],
                                     rhs=xT[:], start=True, stop=False)
                    nc.tensor.matmul(hT_ps[:], lhsT=w1_sb[:, 1, hh * P:(hh + 1) * P],
                                     rhs=aggT[:], start=False, stop=True)
                    nc.scalar.activation(hstore[:, r0:r0 + ST], hT_ps[:],
                                         AF.Relu, bias=b1_sb[:, hh:hh + 1])

                # y2 = h @ W2b rows -> y2own (project-first for layer 2)
                y2T_ps = pp.tile([P, ST], dt.float32, name="y2T_ps", tag="y2T_ps", bufs=1)
                nc.tensor.matmul(y2T_ps[:], lhsT=w2_sb[:, 2, :],
                                 rhs=hta[:, r0:r0 + ST], start=True, stop=False)
                nc.tensor.matmul(y2T_ps[:], lhsT=w2_sb[:, 3, :],
                                 rhs=htb[:, r0:r0 + ST], start=False, stop=True)
                y2T = sp.tile([P, ST], dt.bfloat16, name="y2T", tag="y2T")
                nc.scalar.activation(y2T[:], y2T_ps[:], AF.Copy)
                for nh in range(2):
                    rr = r0 + nh * P
                    nrows = min(P, NPC - rr)
                    if nrows <= 0:
                        continue
                    y2r_ps = pp.tile([P, P], dt.bfloat16, name="y2r_ps",
                                     tag="y2r_ps", bufs=1)
                    nc.tensor.transpose(y2r_ps[:], y2T[:, nh * P:(nh + 1) * P],
                                        ident_bf[:])
                    y2r = sp.tile([P, P], dt.bfloat16, name="y2r", tag="y2r", bufs=3)
                    nc.scalar.activation(y2r[:], y2r_ps[:], AF.Copy)
                    # dest row for node j: j + (NZ if j >= BLK else 0)
                    if rr + nrows <= BLK:
                        nc.sync.dma_start(y2own[rr:rr + nrows, :], y2r[0:nrows, :])
                    elif rr >= BLK:
                        nc.sync.dma_start(y2own[rr + NZ:rr + NZ + nrows, :],
                                          y2r[0:nrows, :])
                    else:
                        n1 = BLK - rr
                        nc.sync.dma_start(y2own[rr:BLK, :], y2r[0:n1, :])
                        nc.sync.dma_start(y2own[BLK + NZ:rr + NZ + nrows, :],
                                          y2r[n1:nrows, :])

            def _l2_supertile(st, gat, co, OVC, dv_sb, id_sb, sp, pp):
                r0 = st * ST
                for sub in range(2):
                    rr = r0 + sub * P
                    agg_ps = _agg_psum(gat, co, sub, OVC, dv_sb, st, pp, sp)
                    agg_sb = sp.tile([P, P], dt.bfloat16, name="agg2_sb",
                                     tag="agg2_sb", bufs=3)
                    nc.scalar.activation(
                        agg_sb[:], agg_ps[:], AF.Copy,
                        scale=id_sb[:, st * 2 + sub:st * 2 + sub + 1])
                    out_ps = pp.tile([P, OUT_DIM], dt.float32, name="out_ps",
                                     tag="out_ps")
                    nc.tensor.matmul(out_ps[:], lhsT=hta[:, rr:rr + P],
                                     rhs=w2_sb[:, 0, :], start=True, stop=False)
                    nc.tensor.matmul(out_ps[:], lhsT=htb[:, rr:rr + P],
                                     rhs=w2_sb[:, 1, :], start=False, stop=False)
                    nc.tensor.matmul(out_ps[:], lhsT=ident_bf[:],
                                     rhs=agg_sb[:], start=False, stop=False)
                    nc.tensor.matmul(out_ps[:], lhsT=ones_1[:],
                                     rhs=b2r_sb[:], start=False, stop=True)
                    o_sb = sp.tile([P, OUT_DIM], dt.float32, name="o_sb",
                                   tag="o_sb", bufs=3)
                    nc.scalar.activation(o_sb[:], out_ps[:], AF.Relu)
                    nc.sync.dma_start(t_out.ap()[rr:rr + P, :], o_sb[:])

            layer(1, NCH1, OVC1, idx1_sb, dv1_sb, id1_sb,
                  [t_xtab2.ap()[h * TB1:h * TB1 + TB1, :] for h in range(2)],
                  P)
            layer(2, NCH2, OVC2, idx2_sb, dv2_sb, id2_sb,
                  [y2full[h][:] for h in range(2)], P)

    nc.compile()
    _PROGRAM_CACHE[key] = nc
    return nc


# ----------------------------------------------------------------------------
# entry point
# ----------------------------------------------------------------------------

def kernel(x, W1, b1, W2, b2, edge_src0, edge_dst0, edge_src1, edge_dst1,
           _want_results=False, **_ignored):
    OVC1, OVC2, in_maps = _preprocess(x, W1, b1, W2, b2,
                                      edge_src0, edge_dst0, edge_src1, edge_dst1)
    nc = build_program(OVC1, OVC2)
    res = run_bass_kernel_spmd(nc, in_maps, core_ids=list(range(NCORES)))
    out = np.concatenate([res.results[c]["out"][:NPC] for c in range(NCORES)], axis=0)
    out = np.ascontiguousarray(out, dtype=np.float32)
    if _want_results:
        return out, res
    return out


# revision 17
# speedup vs baseline: 2.0716x; 1.0517x over previous
"""GraphSAGE 2-layer encoder on 8 Trainium2 NeuronCores (Bass/Tile).

v2 strategy (vs v1 baseline at ~1.18ms):
  The v1 profile showed the Vector engine as the bottleneck (952us active,
  81%): one DVE tensor_scalar per 128-edge chunk built a weighted one-hot
  "sel" matrix (1800 builds/core). v2 eliminates almost all of that:

  - Fixed-slot aggregation: every dst node gets D0=8 gather slots per
    source-half. Slot->dst mapping is then compile-time constant, so the
    selection matrices are EIGHT constant [128,128] one-hots built once.
    Unused slots gather a dedicated zero row appended to the table.
    Degree normalization (1/deg) is applied afterwards as a per-partition
    scale in the scalar-engine PSUM->SBUF copy (agg kept node-major).
  - Overflow edges (per-(dst,half) degree > 8, ~14% of edges) go through
    per-(subtile,half) chunks with DVE-built unweighted one-hots
    (~200 DVE ops/layer instead of 1800 weighted ones).
  - Layer-2 "project-first": y2 = h @ W2[256:] is computed per-core
    ([*,128] bf16), AllGathered instead of h (halves collective bytes),
    and mean-aggregated directly: agg2@W2b == (M2 h)@W2b == M2 y2.
    The output PSUM accumulates self-term + identity*agg2_scaled + bias
    with no transposes.
  - Gathers are issued per (supertile-pair, half) with ~5120 indices each
    (the SWDGE ring holds num_idxs/16+1 entries, cap 2048), cutting Q7
    prep overhead ~4x vs v1's 300 gathers.

  Nodes are range-sharded (6250/core, padded to 6400); per-edge source
  rows are fetched with gpsimd dma_gather (int16 indices -> tables split
  into two <32768-row halves, each with a trailing zero row).
"""

import dataclasses

import numpy as np
import ml_dtypes

import concourse.bass as bass
import concourse.mybir as mybir
import concourse.tile as tile
from concourse import bacc
from concourse.bass_utils import run_bass_kernel_spmd
from concourse.masks import make_identity

BF16 = ml_dtypes.bfloat16

# problem constants (hardcoded per contract)
N = 50000
E = 800000
IN_DIM = 128
HID = 256
OUT_DIM = 128

NCORES = 8
NPC = N // NCORES          # 6250 nodes per core
ST = 256                   # supertile (dst nodes per outer iteration)
NST = 25                   # supertiles per core (6400 padded rows)
NPAD = NST * ST            # 6400
P = 128

D0 = 8                     # main slots per (dst, source-half)
MAINCH = 16                # main chunks per (st, half): 2 subs x 8
HALF1 = N // 2             # 25000 rows per x-table half
BLK = NPC // 2             # 3125 allgather data rows per core per chunk
NZ = 16                    # zero rows appended per core allgather block
CBLK = BLK + NZ            # 3141
HALF2 = NCORES * CBLK      # 25128 rows per y2 table half
NZ1 = 128                  # zero rows appended per layer-1 x-table half
TB1 = HALF1 + NZ1          # 25128
OVPAD = 200.0              # out-of-range dst sentinel for overflow pads

# gather groups: supertile 0 alone, then pairs (1,2)...(23,24)
GROUPS = [[0]] + [[2 * i + 1, 2 * i + 2] for i in range(12)]

_PROGRAM_CACHE: dict = {}


# ----------------------------------------------------------------------------
# host-side preprocessing
# ----------------------------------------------------------------------------

def _pack_layer(src, dst, rel, half, zrows):
    """Build fixed-slot gather tables for one layer.

    rel: int16 table row (within half) per edge's src; half: 0/1 per edge.
    Returns (OVC, idx_tabs, dstov_tabs, invdeg_tabs); per core:
      idx_tab [128, NST*2*NCH*8] int16   (NCH = 16 + 2*OVC)
      dstov   [128, NST*2*2*OVC] float32 (within-sub dst of ovf slot, or 200)
      invdeg  [128, NST*2]       float32
    Instruction-order layout: blocks (group g, half h), within a block
    supertiles ascending, chunks 0..NCH-1, 128 partitions; linear slot t
    wraps to (partition t%16, col t//16), tiled x8 to 128 partitions.
    """
    deg = np.bincount(dst, minlength=N)

    # rank of each edge within its (dst, half) bucket
    key = dst * 2 + half
    order = np.argsort(key, kind="stable")
    counts = np.bincount(key, minlength=2 * N)
    starts = np.concatenate([[0], np.cumsum(counts)])
    rank = np.empty(E, dtype=np.int64)
    rank[order] = np.arange(E) - starts[key[order]]

    core = dst // NPC
    q = dst - core * NPC
    st = q >> 8
    i = q & 255
    sub = i >> 7
    ii = i & 127

    is_main = rank < D0
    # ---- main slots ----
    m_chunk = sub[is_main] * D0 + (ii[is_main] >> 4)        # 0..15
    m_part = (ii[is_main] & 15) * D0 + rank[is_main]        # 0..127
    m_core, m_st, m_half = core[is_main], st[is_main], half[is_main]
    m_rel = rel[is_main]

    # ---- overflow ----
    ov = ~is_main
    ogid = ((core[ov] * NST + st[ov]) * 2 + sub[ov]) * 2 + half[ov]
    ngroups = NCORES * NST * 4
    oorder = np.argsort(ogid, kind="stable")
    ocounts = np.bincount(ogid, minlength=ngroups)
    OVC = max(1, int(-(-ocounts.max() // P)))
    ostarts = np.concatenate([[0], np.cumsum(ocounts)])
    opos = np.empty(ov.sum(), dtype=np.int64)
    opos[oorder] = np.arange(ov.sum()) - ostarts[ogid[oorder]]
    o_chunk = MAINCH + sub[ov] * OVC + (opos >> 7)          # within (st,half)
    o_part = opos & 127
    o_core, o_st, o_half = core[ov], st[ov], half[ov]
    o_rel, o_ii = rel[ov], ii[ov]

    NCH = MAINCH + 2 * OVC

    # idx array in (core, st, half, chunk, part) space; pads spread over
    # many distinct zero rows so they don't hotspot one HBM line
    idx = np.ascontiguousarray(np.broadcast_to(
        zrows.astype(np.int16), (NCORES, NST, 2, NCH, P)))
    idx[m_core, m_st, m_half, m_chunk, m_part] = m_rel
    idx[o_core, o_st, o_half, o_chunk, o_part] = o_rel

    # [core, part, st, sub, half, ovc]; (o_chunk-MAINCH)//OVC == sub
    dstov = np.full((NCORES, P, NST, 2, 2, OVC), OVPAD, dtype=np.float32)
    dstov[o_core, o_part, o_st, (o_chunk - MAINCH) // OVC,
          o_half, (o_chunk - MAINCH) % OVC] = o_ii

    invdeg = np.zeros((NCORES, P, NST, 2), dtype=np.float32)
    nodes = np.arange(N)
    invdeg[nodes // NPC, (nodes % NPC) & 127, (nodes % NPC) >> 8,
           ((nodes % NPC) & 255) >> 7] = 1.0 / np.maximum(deg, 1.0)

    # reorder idx into instruction-order column layout with 16-partition wrap
    idx_tabs, dstov_tabs, invdeg_tabs = [], [], []
    for c in range(NCORES):
        blocks = []
        for g in GROUPS:
            for h in range(2):
                # [len(g), NCH, P] -> linear t = (st_loc*NCH + ch)*128 + p
                lin = idx[c, g, h].reshape(-1)               # len(g)*NCH*128
                w = lin.reshape(-1, 16).T                     # [16, t//16]
                blocks.append(w)
        it = np.concatenate(blocks, axis=1)                   # [16, total]
        idx_tabs.append(np.ascontiguousarray(np.tile(it, (8, 1))))
        dstov_tabs.append(np.ascontiguousarray(
            dstov[c].reshape(P, NST * 2 * 2 * OVC)))
        invdeg_tabs.append(np.ascontiguousarray(
            invdeg[c].reshape(P, NST * 2)))
    return OVC, idx_tabs, dstov_tabs, invdeg_tabs


def _preprocess(x, W1, b1, W2, b2, es0, ed0, es1, ed1):
    x = np.asarray(x, dtype=np.float32)
    es0 = np.asarray(es0, dtype=np.int64)
    ed0 = np.asarray(ed0, dtype=np.int64)
    es1 = np.asarray(es1, dtype=np.int64)
    ed1 = np.asarray(ed1, dtype=np.int64)

    # layer 1: table = x split in two halves, each with trailing zero rows
    p_ar = np.arange(P)
    zr1 = HALF1 + (p_ar % NZ1)
    OVC1, idx1, dv1, id1 = _pack_layer(
        es0, ed0, (es0 % HALF1).astype(np.int16), es0 // HALF1, zr1)

    # layer 2: y2 table rows: node n=c*NPC+j -> half j//BLK, row c*CBLK+j%BLK
    c_of = es1 // NPC
    j_of = es1 % NPC
    zr2 = (p_ar % NCORES) * CBLK + BLK + ((p_ar // NCORES) % NZ)
    OVC2, idx2, dv2, id2 = _pack_layer(
        es1, ed1, (c_of * CBLK + (j_of % BLK)).astype(np.int16), j_of // BLK, zr2)

    x_bf = x.astype(BF16)
    xtab2 = np.zeros((2 * TB1, IN_DIM), dtype=BF16)
    xtab2[0:HALF1] = x_bf[0:HALF1]
    xtab2[TB1:TB1 + HALF1] = x_bf[HALF1:N]

    xown = []
    for c in range(NCORES):
        xo = np.zeros((NPAD, IN_DIM), dtype=BF16)
        xo[:NPC] = x_bf[c * NPC:(c + 1) * NPC]
        xown.append(xo)

    W1_bf = np.asarray(W1, np.float32).astype(BF16)           # [256, 256]
    W2_bf = np.asarray(W2, np.float32).astype(BF16)           # [512, 128]
    b1_2 = np.asarray(b1, np.float32).reshape(2, P).T.copy()  # [128, 2]
    b2_r = np.asarray(b2, np.float32).reshape(1, P).astype(BF16)

    # scol[p, c] = c*16 + p//8 (selection one-hot compare values)
    scol = (np.arange(P)[:, None] // D0
            + np.arange(8)[None, :] * 16.0).astype(np.float32)

    in_maps = []
    for c in range(NCORES):
        in_maps.append({
            "xtab2": xtab2,
            "xown": xown[c],
            "w1": W1_bf,
            "w2": W2_bf,
            "b1": b1_2,
            "b2r": b2_r,
            "scol": scol,
            "idx1": idx1[c], "dv1": dv1[c], "id1": id1[c],
            "idx2": idx2[c], "dv2": dv2[c], "id2": id2[c],
        })
    return OVC1, OVC2, in_maps


# ----------------------------------------------------------------------------
# device program
# ----------------------------------------------------------------------------

def build_program(OVC1, OVC2, ablate=()):
    key = (OVC1, OVC2, tuple(sorted(ablate)))
    if key in _PROGRAM_CACHE:
        return _PROGRAM_CACHE[key]

    NCH1 = MAINCH + 2 * OVC1
    NCH2 = MAINCH + 2 * OVC2
    dt = mybir.dt
    AF = mybir.ActivationFunctionType
    nc = bacc.Bacc("TRN2", target_bir_lowering=False, debug=False,
                   num_devices=NCORES, num_swdge_queues=4,
                   dynamic_dma_scratch_size=32768)

    t_xtab2 = nc.dram_tensor("xtab2", [2 * TB1, IN_DIM], dt.bfloat16, kind="ExternalInput")
    t_xown = nc.dram_tensor("xown", [NPAD, IN_DIM], dt.bfloat16, kind="ExternalInput")
    t_w1 = nc.dram_tensor("w1", [HID, HID], dt.bfloat16, kind="ExternalInput")
    t_w2 = nc.dram_tensor("w2", [2 * HID, OUT_DIM], dt.bfloat16, kind="ExternalInput")
    t_b1 = nc.dram_tensor("b1", [P, 2], dt.float32, kind="ExternalInput")
    t_b2r = nc.dram_tensor("b2r", [1, OUT_DIM], dt.bfloat16, kind="ExternalInput")
    t_scol = nc.dram_tensor("scol", [P, 8], dt.float32, kind="ExternalInput")
    t_idx1 = nc.dram_tensor("idx1", [P, NST * 2 * NCH1 * 8], dt.int16, kind="ExternalInput")
    t_dv1 = nc.dram_tensor("dv1", [P, NST * 2 * 2 * OVC1], dt.float32, kind="ExternalInput")
    t_id1 = nc.dram_tensor("id1", [P, NST * 2], dt.float32, kind="ExternalInput")
    t_idx2 = nc.dram_tensor("idx2", [P, NST * 2 * NCH2 * 8], dt.int16, kind="ExternalInput")
    t_dv2 = nc.dram_tensor("dv2", [P, NST * 2 * 2 * OVC2], dt.float32, kind="ExternalInput")
    t_id2 = nc.dram_tensor("id2", [P, NST * 2], dt.float32, kind="ExternalInput")
    t_out = nc.dram_tensor("out", [NPAD, OUT_DIM], dt.float32, kind="ExternalOutput")

    qctr = [0]

    def gather(gat_h, in_ap, idx_sb, col0, ncols, num_idxs, ew):
        nc.gpsimd.dma_gather(
            out_ap=gat_h,
            in_ap=in_ap,
            idxs_ap=idx_sb[:, col0:col0 + ncols],
            num_idxs=num_idxs,
            num_idxs_reg=num_idxs,
            elem_size=ew,
            elem_step=P if ew != P else None,
            queue_num=qctr[0] % 4,
        )
        qctr[0] += 1

    with tile.TileContext(nc) as tc:
        with tc.tile_pool(name="const", bufs=1) as cp, \
             tc.tile_pool(name="dram", bufs=1, space="DRAM") as dp:

            # ---- constants / persistent SBUF ----
            ident_bf = cp.tile([P, P], dt.bfloat16, name="ident_bf")
            make_identity(nc, ident_bf)
            iota_i = cp.tile([P, P], dt.int32, name="iota_i")
            nc.gpsimd.iota(iota_i, pattern=[[1, P]], base=0, channel_multiplier=0)
            iota_bf = cp.tile([P, P], dt.bfloat16, name="iota_bf")
            nc.vector.tensor_copy(iota_bf[:], iota_i[:])
            ones_1 = cp.tile([1, P], dt.bfloat16, name="ones_1")
            nc.vector.memset(ones_1[:], 1.0)
            zrow_bf = cp.tile([NZ, P], dt.bfloat16, name="zrow_bf")
            nc.vector.memset(zrow_bf[:], 0.0)
            b2r_sb = cp.tile([1, OUT_DIM], dt.bfloat16, name="b2r_sb")

            w1_sb = cp.tile([P, 2, HID], dt.bfloat16, name="w1_sb")
            nc.sync.dma_start(w1_sb[:], t_w1.ap().rearrange("(a p) h -> p a h", p=P))
            w2_sb = cp.tile([P, 4, OUT_DIM], dt.bfloat16, name="w2_sb")
            nc.sync.dma_start(w2_sb[:], t_w2.ap().rearrange("(a p) h -> p a h", p=P))
            b1_sb = cp.tile([P, 2], dt.float32, name="b1_sb")
            nc.sync.dma_start(b1_sb[:], t_b1.ap()[:])
            nc.sync.dma_start(b2r_sb[:], t_b2r.ap()[:])
            scol_sb = cp.tile([P, 8], dt.float32, name="scol_sb")
            nc.sync.dma_start(scol_sb[:], t_scol.ap()[:])

            idx1_sb = cp.tile([P, NST * 2 * NCH1 * 8], dt.int16, name="idx1_sb")
            nc.sync.dma_start(idx1_sb[:], t_idx1.ap()[:])
            dv1_sb = cp.tile([P, NST * 2 * 2 * OVC1], dt.float32, name="dv1_sb")
            nc.sync.dma_start(dv1_sb[:], t_dv1.ap()[:])
            id1_sb = cp.tile([P, NST * 2], dt.float32, name="id1_sb")
            nc.sync.dma_start(id1_sb[:], t_id1.ap()[:])
            idx2_sb = cp.tile([P, NST * 2 * NCH2 * 8], dt.int16, name="idx2_sb")
            nc.sync.dma_start(idx2_sb[:], t_idx2.ap()[:])
            dv2_sb = cp.tile([P, NST * 2 * 2 * OVC2], dt.float32, name="dv2_sb")
            nc.sync.dma_start(dv2_sb[:], t_dv2.ap()[:])
            id2_sb = cp.tile([P, NST * 2], dt.float32, name="id2_sb")
            nc.sync.dma_start(id2_sb[:], t_id2.ap()[:])

            # 8 constant selection one-hots: sel8[c][s, d] = (d == c*16 + s//8)
            sel8 = cp.tile([P, 8, P], dt.bfloat16, name="sel8")
            for c in range(8):
                nc.vector.tensor_scalar(
                    out=sel8[:, c, :], in0=iota_bf[:],
                    scalar1=scol_sb[:, c:c + 1], scalar2=None,
                    op0=mybir.AluOpType.is_equal,
                )

            # persistent transposed h (self-features for layer 2)
            hta = cp.tile([P, NPAD], dt.bfloat16, name="hta")
            htb = cp.tile([P, NPAD], dt.bfloat16, name="htb")

            # allgather buffers (y2 = h @ W2b, project-first); each core's
            # send block carries a trailing zero row so the gathered table
            # gets its pad target without a second writer on the Shared side
            y2own = dp.tile([2 * CBLK, OUT_DIM], dt.bfloat16, name="y2own")
            y2full = [
                dp.tile([HALF2, OUT_DIM], dt.bfloat16, name=f"y2full{k}",
                        addr_space="Shared")
                for k in range(2)
            ]
            for k in range(2):
                nc.sync.dma_start(y2own[k * CBLK + BLK:(k + 1) * CBLK, :],
                                  zrow_bf[:])

            # ---- layer 1 ----
            idx_off = [0]

            def layer(lnum, NCH, OVC, idx_sb, dv_sb, id_sb, in_aps, ew):
                """Emit one layer's supertile loop. Returns nothing."""
                with tc.tile_pool(name=f"l{lnum}sb", bufs=2) as sp, \
                     tc.tile_pool(name=f"l{lnum}ps", bufs=2, space="PSUM") as pp:
                    off = 0
                    for g in GROUPS:
                        nst_g = len(g)
                        nidx = nst_g * NCH * P
                        ncols = nidx // 16
                        tag = f"gat{lnum}_{nst_g}"
                        gat = sp.tile([P, 2, nst_g * NCH, ew], dt.bfloat16,
                                      name=tag, tag=tag, bufs=3)
                        for h in range(2):
                            if "gather" in ablate:
                                nc.vector.memset(gat[:, h], 0.0)
                                continue
                            # HW caps one dma_gather at 1024 indices
                            # (16 engines x 64-descriptor rings)
                            nch_all = nst_g * NCH
                            for w0 in range(0, nch_all, 8):
                                wn = min(8, nch_all - w0)
                                gather(gat[:, h, w0:w0 + wn, :], in_aps[h],
                                       idx_sb, off + h * ncols + w0 * 8,
                                       wn * 8, wn * P, ew)
                        for sti, st in enumerate(g):
                            co = sti * NCH
                            if lnum == 1:
                                _l1_supertile(st, gat, co, OVC, dv_sb, id_sb, sp, pp)
                            else:
                                _l2_supertile(st, gat, co, OVC, dv_sb, id_sb, sp, pp)
                        off += 2 * ncols
                        if lnum == 1 and g[-1] == 12:
                            _allgather(0)
                    if lnum == 1:
                        _allgather(1)

            def _allgather(k):
                nc.gpsimd.collective_compute(
                    "AllGather",
                    mybir.AluOpType.bypass,
                    replica_groups=[list(range(NCORES))],
                    ins=[y2own[k * CBLK:(k + 1) * CBLK, :].opt()],
                    outs=[y2full[k][:].opt()],
                )

            def _agg_psum(gat, co, sub, OVC, dv_sb, st, pp, sp):
                """Accumulate main+overflow into a [128dst, 128f] PSUM tile."""
                agg_ps = pp.tile([P, P], dt.float32, name="agg_ps",
                                 tag="agg_ps", bufs=2)
                for c in range(8):
                    for h in range(2):
                        nc.tensor.matmul(
                            agg_ps[:],
                            lhsT=sel8[:, c, :],
                            rhs=gat[:, h, co + sub * 8 + c, 0:P],
                            start=(c == 0 and h == 0), stop=False)
                n_ov = 2 * OVC
                kk = 0
                for h in range(2):
                    for o in range(OVC):
                        col = ((st * 2 + sub) * 2 + h) * OVC + o
                        if "sel" in ablate:
                            sel = sel8[:, 0, :]
                        else:
                            sel = sp.tile([P, P], dt.bfloat16, name="sel_ov",
                                          tag="sel_ov", bufs=8)
                            nc.vector.tensor_scalar(
                                out=sel[:], in0=iota_bf[:],
                                scalar1=dv_sb[:, col:col + 1], scalar2=None,
                                op0=mybir.AluOpType.is_equal,
                            )
                        nc.tensor.matmul(
                            agg_ps[:],
                            lhsT=sel if "sel" in ablate else sel[:],
                            rhs=gat[:, h, co + MAINCH + sub * OVC + o, 0:P],
                            start=False, stop=(kk == n_ov - 1))
                        kk += 1
                return agg_ps

            def _l1_supertile(st, gat, co, OVC, dv_sb, id_sb, sp, pp):
                r0 = st * ST
                # own features, transposed (self term)
                xo = sp.tile([P, 2, IN_DIM], dt.bfloat16, name="xo", tag="xo", bufs=3)
                nc.sync.dma_start(
                    xo[:], t_xown.ap()[r0:r0 + ST, :].rearrange("(a p) f -> p a f", p=P))
                xT_ps = pp.tile([P, ST], dt.bfloat16, name="xT_ps", tag="xT_ps", bufs=1)
                nc.tensor.transpose(xT_ps[:, 0:P], xo[:, 0, :], ident_bf[:])
                nc.tensor.transpose(xT_ps[:, P:ST], xo[:, 1, :], ident_bf[:])
                xT = sp.tile([P, ST], dt.bfloat16, name="xT", tag="xT")
                nc.scalar.activation(xT[:], xT_ps[:], AF.Copy)

                # aggregation, node-major, then scale by 1/deg and transpose
                aggT_ps = pp.tile([P, ST], dt.bfloat16, name="aggT_ps", tag="aggT_ps", bufs=1)
                for sub in range(2):
                    agg_ps = _agg_psum(gat, co, sub, OVC, dv_sb, st, pp, sp)
                    agg_sb = sp.tile([P, P], dt.bfloat16, name="agg_sb",
                                     tag="agg_sb", bufs=3)
                    nc.scalar.activation(
                        agg_sb[:], agg_ps[:], AF.Copy,
                        scale=id_sb[:, st * 2 + sub:st * 2 + sub + 1])
                    nc.tensor.transpose(aggT_ps[:, sub * P:(sub + 1) * P],
                                        agg_sb[:], ident_bf[:])
                aggT = sp.tile([P, ST], dt.bfloat16, name="aggT", tag="aggT")
                nc.scalar.activation(aggT[:], aggT_ps[:], AF.Copy)

                # hT = relu(W1^T @ [x; agg] + b1), two hid halves
                for hh, hstore in ((0, hta), (1, htb)):
                    hT_ps = pp.tile([P, ST], dt.float32, name="hT_ps", tag="hT_ps")
                    nc.tensor.matmul(hT_ps[:], lhsT=w1_sb[:, 0, hh * P:(hh + 1) * P],
                                     rhs=xT[:], start=True, stop=False)
                    nc.tensor.matmul(hT_ps[:], lhsT=w1_sb[:, 1, hh * P:(hh + 1) * P],
                                     rhs=aggT[:], start=False, stop=True)
                    nc.scalar.activation(hstore[:, r0:r0 + ST], hT_ps[:],
                                         AF.Relu, bias=b1_sb[:, hh:hh + 1])

                # y2 = h @ W2b rows -> y2own (project-first for layer 2)
                y2T_ps = pp.tile([P, ST], dt.float32, name="y2T_ps", tag="y2T_ps", bufs=1)
                nc.tensor.matmul(y2T_ps[:], lhsT=w2_sb[:, 2, :],
                                 rhs=hta[:, r0:r0 + ST], start=True, stop=False)
                nc.tensor.matmul(y2T_ps[:], lhsT=w2_sb[:, 3, :],
                                 rhs=htb[:, r0:r0 + ST], start=False, stop=True)
                y2T = sp.tile([P, ST], dt.bfloat16, name="y2T", tag="y2T")
                nc.scalar.activation(y2T[:], y2T_ps[:], AF.Copy)
                for nh in range(2):
                    rr = r0 + nh * P
                    nrows = min(P, NPC - rr)
                    if nrows <= 0:
                        continue
                    y2r_ps = pp.tile([P, P], dt.bfloat16, name="y2r_ps",
                                     tag="y2r_ps", bufs=1)
                    nc.tensor.transpose(y2r_ps[:], y2T[:, nh * P:(nh + 1) * P],
                                        ident_bf[:])
                    y2r = sp.tile([P, P], dt.bfloat16, name="y2r", tag="y2r", bufs=3)
                    nc.scalar.activation(y2r[:], y2r_ps[:], AF.Copy)
                    # dest row for node j: j + (NZ if j >= BLK else 0)
                    if rr + nrows <= BLK:
                        nc.sync.dma_start(y2own[rr:rr + nrows, :], y2r[0:nrows, :])
                    elif rr >= BLK:
                        nc.sync.dma_start(y2own[rr + NZ:rr + NZ + nrows, :],
                                          y2r[0:nrows, :])
                    else:
                        n1 = BLK - rr
                        nc.sync.dma_start(y2own[rr:BLK, :], y2r[0:n1, :])
                        nc.sync.dma_start(y2own[BLK + NZ:rr + NZ + nrows, :],
                                          y2r[n1:nrows, :])

            def _l2_supertile(st, gat, co, OVC, dv_sb, id_sb, sp, pp):
                r0 = st * ST
                for sub in range(2):
                    rr = r0 + sub * P
                    agg_ps = _agg_psum(gat, co, sub, OVC, dv_sb, st, pp, sp)
                    agg_sb = sp.tile([P, P], dt.bfloat16, name="agg2_sb",
                                     tag="agg2_sb", bufs=3)
                    nc.scalar.activation(
                        agg_sb[:], agg_ps[:], AF.Copy,
                        scale=id_sb[:, st * 2 + sub:st * 2 + sub + 1])
                    out_ps = pp.tile([P, OUT_DIM], dt.float32, name="out_ps",
                                     tag="out_ps")
                    nc.tensor.matmul(out_ps[:], lhsT=hta[:, rr:rr + P],
                                     rhs=w2_sb[:, 0, :], start=True, stop=False)
                    nc.tensor.matmul(out_ps[:], lhsT=htb[:, rr:rr + P],
                                     rhs=w2_sb[:, 1, :], start=False, stop=False)
                    nc.tensor.matmul(out_ps[:], lhsT=ident_bf[:],
                                     rhs=agg_sb[:], start=False, stop=False)
                    nc.tensor.matmul(out_ps[:], lhsT=ones_1[:],
                                     rhs=b2r_sb[:], start=False, stop=True)
                    o_sb = sp.tile([P, OUT_DIM], dt.float32, name="o_sb",
                                   tag="o_sb", bufs=3)
                    nc.scalar.activation(o_sb[:], out_ps[:], AF.Relu)
                    nc.sync.dma_start(t_out.ap()[rr:rr + P, :], o_sb[:])

            layer(1, NCH1, OVC1, idx1_sb, dv1_sb, id1_sb,
                  [t_xtab2.ap()[h * TB1:h * TB1 + TB1, :] for h in range(2)],
                  P)
            layer(2, NCH2, OVC2, idx2_sb, dv2_sb, id2_sb,
                  [y2full[h][:] for h in range(2)], P)

    nc.compile()
    _PROGRAM_CACHE[key] = nc
    return nc


# ----------------------------------------------------------------------------
# entry point
# ----------------------------------------------------------------------------

def kernel(x, W1, b1, W2, b2, edge_src0, edge_dst0, edge_src1, edge_dst1,
           _want_results=False, **_ignored):
    OVC1, OVC2, in_maps = _preprocess(x, W1, b1, W2, b2,
                                      edge_src0, edge_dst0, edge_src1, edge_dst1)
    nc = build_program(OVC1, OVC2)
    res = run_bass_kernel_spmd(nc, in_maps, core_ids=list(range(NCORES)))
    out = np.concatenate([res.results[c]["out"][:NPC] for c in range(NCORES)], axis=0)
    out = np.ascontiguousarray(out, dtype=np.float32)
    if _want_results:
        return out, res
    return out
